# revision 13
# baseline (speedup 1.0000x reference)
"""Trainium2 Bass kernel for nn_ConditionedDense (hypernetwork-conditioned dense).

Reference computation:
    A = einsum('bnp,pq->bnq', P, Wk)         # hypernetwork: per-position weights
    W = relu(A).reshape(B, N, c_in, c_out)
    out = einsum('bni,bnio->bno', X, W)

Strategy (v4): pure data parallel over 8 NeuronCores (shard batch dim),
A^T-oriented dataflow so both einsums run on the PE with static weights:

  - A^T layout: [q' partitions, pos free] with q' = o*32 + i.  PE computes
    A^T chunks (128 q' x T pos) with lhsT = Wk' chunk (static), rhs = P^T.
    K=64 -> two chunks run concurrently via row tiling (rows 0-63 / 64-127),
    with P^T duplicated on partitions 64-127.
  - m = relu(A) * X, per-tile path choice to balance ACT and DVE:
      D tile: 4x scalar.activation(Relu) PSUM->SBUF bf16 into one w8 tile,
              then ONE DVE tensor_tensor mult (2x bf16, FD=4096) by X
              replicated 4x on partitions (X_rep[p,t] = X[t, p%32]).
      S tile: 4x fused DVE scalar_tensor_tensor (max 0, mult) from PSUM.
  - reduce over i on the PE: 8 accumulating matmuls per tile with static
    0/1 selection weights S_c[p, o] = (o == 4c + p//32), output col-tiled
    into out^T PSUM [32j:32j+32, :].  Tiles are processed in PAIRS with
    their reduce matmuls emitted adjacently -> different col groups run
    concurrently on the PE array.
  - out stays in packed transposed layout; host unpacks (free).

Host side (free): P^T duplicated x2, X^T replicated x4, Wk column-permuted
to q' = o*32+i and packed into row-tiled pairs, S selection matrices, all
cast to bf16.
"""

import os
from contextlib import ExitStack

import numpy as np
import ml_dtypes

import concourse.bass as bass
import concourse.tile as tile
from concourse import bacc, mybir
from concourse.bass_utils import run_bass_kernel_spmd

C_IN = 32
C_OUT = 32
P_DIM = 64
Q = C_IN * C_OUT             # 1024
B, N = 32, 4096
N_CORES = 8
B_SH = B // N_CORES          # 4 batches per core
NPOS = B_SH * N              # 16384 positions per core
T = 512                      # positions per tile (matmul N)
TILES = NPOS // T            # 32
SG_TILES = 4                 # tiles per supergroup (col-tiled out^T group)
N_SG = TILES // SG_TILES     # 8
T_SG = T * SG_TILES          # 2048 positions per supergroup
PAIRS = 4                    # chunk pairs per tile (8 q'-chunks of 128)
# per chunk-pair-unit m-production path, indexed by unit_idx % 32:
#   D = ACT relu -> DVE tensor_tensor mult (2x bf16)
#   S = fused DVE scalar_tensor_tensor (relu+mult) straight from PSUM
UNIT_PATHS = os.environ.get(
    "K_PATHS", "SDDSDDDSDDSDDDSDDDSDDSDDDSDDSDDD"
)

F32 = mybir.dt.float32
BF16 = mybir.dt.bfloat16

_BUILD_CACHE = {}
LAST_RESULTS = None  # BassKernelResults of the most recent run (for profiling)


def _build_nc():
    nc = bacc.Bacc(
        "TRN2", target_bir_lowering=False, debug=False, num_devices=N_CORES
    )
    XR_d = nc.declare_dram_parameter("XR", [N_SG * 128, T_SG], BF16, isOutput=False)
    P2_d = nc.declare_dram_parameter("P2", [N_SG * 128, T_SG], BF16, isOutput=False)
    WK_d = nc.declare_dram_parameter("WK", [128, PAIRS * 128], BF16, isOutput=False)
    S_d = nc.declare_dram_parameter("S", [128, 8 * C_OUT], BF16, isOutput=False)
    out_d = nc.declare_dram_parameter("out", [N_SG * 128, T], BF16, isOutput=True)

    relu = mybir.ActivationFunctionType.Relu
    copyf = mybir.ActivationFunctionType.Copy
    mult = mybir.AluOpType.mult
    amax = mybir.AluOpType.max

    with ExitStack() as ctx:
        tc = ctx.enter_context(tile.TileContext(nc))
        wkp = ctx.enter_context(tc.tile_pool(name="wk", bufs=1))
        ssp = ctx.enter_context(tc.tile_pool(name="sel", bufs=1))
        xrp = ctx.enter_context(tc.tile_pool(name="xr", bufs=2))
        p2p = ctx.enter_context(tc.tile_pool(name="p2", bufs=2))
        apool = ctx.enter_context(tc.tile_pool(name="apsum", bufs=3, space="PSUM"))
        wpool = ctx.enter_context(tc.tile_pool(name="w", bufs=4))
        mpool = ctx.enter_context(tc.tile_pool(name="m", bufs=6))
        opool = ctx.enter_context(tc.tile_pool(name="opsum", bufs=2, space="PSUM"))
        obp = ctx.enter_context(tc.tile_pool(name="osb", bufs=2))

        wk_t = wkp.tile([128, PAIRS, 128], BF16)
        nc.sync.dma_start(
            out=wk_t[:], in_=WK_d[:].rearrange("p (a b) -> p a b", a=PAIRS)
        )
        s_t = ssp.tile([128, 8, C_OUT], BF16)
        nc.sync.dma_start(
            out=s_t[:], in_=S_d[:].rearrange("p (a b) -> p a b", a=8)
        )

        for sg in range(N_SG):
            xr = xrp.tile([128, T_SG], BF16)
            p2 = p2p.tile([128, T_SG], BF16)
            # split loads along the free dim so tile j only waits on its
            # quarter; split p2 also by partition half (row-tiling operands)
            for fq in range(SG_TILES):
                fs = bass.ts(fq, T)
                r0 = sg * 128
                nc.sync.dma_start(out=xr[:, fs], in_=XR_d[r0:r0 + 128, fs])
                nc.sync.dma_start(
                    out=p2[0:64, fs], in_=P2_d[r0:r0 + 64, fs]
                )
                nc.sync.dma_start(
                    out=p2[64:128, fs], in_=P2_d[r0 + 64:r0 + 128, fs]
                )

            ot = opool.tile([128, T], F32)

            def emit_reduce(p, units):
                # adjacent emission of the two tiles' matmuls -> different
                # col groups run concurrently on the PE
                for c2 in range(2):
                    chunk = 2 * p + c2
                    for (j, m) in units:
                        nc.tensor.matmul(
                            ot[32 * j:32 * (j + 1), :],
                            lhsT=s_t[:, chunk, :], rhs=m[:, c2, :],
                            start=(chunk == 0), stop=(chunk == 7),
                            tile_position=(0, 32 * j),
                        )

            pending = None
            for tp in range(SG_TILES // 2):
                j0, j1 = 2 * tp, 2 * tp + 1
                for p in range(PAIRS):
                    units = []
                    for j in (j0, j1):
                        js = bass.ts(j, T)
                        psA = apool.tile([128, 2, T], F32)
                        nc.tensor.matmul(
                            psA[:, 0, :], lhsT=wk_t[0:64, p, :],
                            rhs=p2[0:64, js], start=True, stop=True,
                        )
                        nc.tensor.matmul(
                            psA[:, 1, :], lhsT=wk_t[64:128, p, :],
                            rhs=p2[64:128, js], start=True, stop=True,
                        )
                        m = mpool.tile([128, 2, T], BF16)
                        x_in = xr[:, js].unsqueeze(1).broadcast_to(
                            [128, 2, T]
                        )
                        u = ((sg * SG_TILES + j) * PAIRS + p) % len(
                            UNIT_PATHS
                        )
                        if UNIT_PATHS[u] == "S":
                            nc.vector.scalar_tensor_tensor(
                                out=m[:], in0=psA[:], scalar=0.0,
                                in1=x_in, op0=amax, op1=mult,
                            )
                        else:
                            w = wpool.tile([128, 2, T], BF16)
                            nc.scalar.activation(w[:], psA[:], relu)
                            nc.vector.tensor_tensor(
                                out=m[:], in0=w[:], in1=x_in, op=mult
                            )
                        units.append((j, m))
                    # lag the reduce by one pair so the PE always has
                    # A-gen work queued ahead of a reduce that may stall
                    if pending is not None:
                        emit_reduce(*pending)
                    pending = (p, units)
            if pending is not None:
                emit_reduce(*pending)
                pending = None

            osb = obp.tile([128, T], BF16)
            nc.scalar.activation(osb[:], ot[:], copyf)
            nc.sync.dma_start(
                out=out_d[sg * 128:(sg + 1) * 128, :], in_=osb[:]
            )

    nc.finalize()
    return nc


def _get_nc():
    key = "v5a-" + UNIT_PATHS
    if key not in _BUILD_CACHE:
        _BUILD_CACHE[key] = _build_nc()
    return _BUILD_CACHE[key]


def _host_prep(X, P, Wk):
    """Build per-core input arrays (host-side prep is free)."""
    bf16 = ml_dtypes.bfloat16
    # Wk' with q' = o*32 + i
    WkP = np.ascontiguousarray(
        Wk.reshape(P_DIM, C_IN, C_OUT).transpose(0, 2, 1).reshape(P_DIM, Q)
    )
    # packed row-tiled pairs: [128, PAIRS, 128] -> [128, PAIRS*128]
    wk2 = np.zeros((128, PAIRS, 128), dtype=np.float32)
    for p in range(PAIRS):
        wk2[0:64, p, :] = WkP[:, 256 * p:256 * p + 128]
        wk2[64:128, p, :] = WkP[:, 256 * p + 128:256 * p + 256]
    WK_h = np.ascontiguousarray(wk2.reshape(128, PAIRS * 128)).astype(bf16)

    # S selection: S[pr, c, o] = 1 if o == 4c + pr//32
    pr = np.arange(128)[:, None, None]
    cc = np.arange(8)[None, :, None]
    oo = np.arange(C_OUT)[None, None, :]
    S = (oo == 4 * cc + pr // 32).astype(np.float32)
    S_h = np.ascontiguousarray(S.reshape(128, 8 * C_OUT)).astype(bf16)

    in_maps = []
    for c in range(N_CORES):
        Xc = np.ascontiguousarray(
            X[c * B_SH:(c + 1) * B_SH].reshape(NPOS, C_IN)
        )
        Pc = np.ascontiguousarray(
            P[c * B_SH:(c + 1) * B_SH].reshape(NPOS, P_DIM)
        )
        # X_rep [128, NPOS]: row pr = X[:, pr % 32]; then [sg] blocks
        XRc = np.tile(Xc.T, (4, 1))                    # [128, NPOS]
        XR_h = np.ascontiguousarray(
            XRc.reshape(128, N_SG, T_SG).transpose(1, 0, 2).reshape(
                N_SG * 128, T_SG
            )
        ).astype(bf16)
        # P^T duplicated x2 on partitions
        P2c = np.tile(Pc.T, (2, 1))                    # [128, NPOS]
        P2_h = np.ascontiguousarray(
            P2c.reshape(128, N_SG, T_SG).transpose(1, 0, 2).reshape(
                N_SG * 128, T_SG
            )
        ).astype(bf16)
        in_maps.append({"XR": XR_h, "P2": P2_h, "WK": WK_h, "S": S_h})
    return in_maps


def kernel(X, P, Wk):
    global LAST_RESULTS
    X = np.asarray(X, dtype=np.float32)
    P = np.asarray(P, dtype=np.float32)
    Wk = np.asarray(Wk, dtype=np.float32)

    in_maps = _host_prep(X, P, Wk)

    nc = _get_nc()
    trace = os.environ.get("BASS_PROFILE", "0") == "1"
    kw = {}
    if os.environ.get("BASS_TMPDIR"):
        kw["tmpdir"] = os.environ["BASS_TMPDIR"]
    res = run_bass_kernel_spmd(
        nc, in_maps, list(range(N_CORES)), trace=trace, **kw
    )
    LAST_RESULTS = res

    out = np.empty((B, N, C_OUT), dtype=np.float32)
    for c in range(N_CORES):
        # packed out^T: [sg*128 + 32j + o, t] -> pos = sg*T_SG + j*T + t
        o_c = (
            np.asarray(res.results[c]["out"])
            .astype(np.float32)
            .reshape(N_SG, SG_TILES, C_OUT, T)
            .transpose(0, 1, 3, 2)
            .reshape(B_SH, N, C_OUT)
        )
        out[c * B_SH:(c + 1) * B_SH] = o_c
    return out


# revision 15
# speedup vs baseline: 1.0532x; 1.0532x over previous
"""Trainium2 Bass kernel for nn_ConditionedDense (hypernetwork-conditioned dense).

Reference computation:
    A = einsum('bnp,pq->bnq', P, Wk)         # hypernetwork: per-position weights
    W = relu(A).reshape(B, N, c_in, c_out)
    out = einsum('bni,bnio->bno', X, W)

Strategy (v4): pure data parallel over 8 NeuronCores (shard batch dim),
A^T-oriented dataflow so both einsums run on the PE with static weights:

  - A^T layout: [q' partitions, pos free] with q' = o*32 + i.  PE computes
    A^T chunks (128 q' x T pos) with lhsT = Wk' chunk (static), rhs = P^T.
    K=64 -> two chunks run concurrently via row tiling (rows 0-63 / 64-127),
    with P^T duplicated on partitions 64-127.
  - m = relu(A) * X, per-tile path choice to balance ACT and DVE:
      D tile: 4x scalar.activation(Relu) PSUM->SBUF bf16 into one w8 tile,
              then ONE DVE tensor_tensor mult (2x bf16, FD=4096) by X
              replicated 4x on partitions (X_rep[p,t] = X[t, p%32]).
      S tile: 4x fused DVE scalar_tensor_tensor (max 0, mult) from PSUM.
  - reduce over i on the PE: 8 accumulating matmuls per tile with static
    0/1 selection weights S_c[p, o] = (o == 4c + p//32), output col-tiled
    into out^T PSUM [32j:32j+32, :].  Tiles are processed in PAIRS with
    their reduce matmuls emitted adjacently -> different col groups run
    concurrently on the PE array.
  - out stays in packed transposed layout; host unpacks (free).

Host side (free): P^T duplicated x2, X^T replicated x4, Wk column-permuted
to q' = o*32+i and packed into row-tiled pairs, S selection matrices, all
cast to bf16.
"""

import os
from contextlib import ExitStack

import numpy as np
import ml_dtypes

import concourse.bass as bass
import concourse.tile as tile
from concourse import bacc, mybir
from concourse.bass_utils import run_bass_kernel_spmd

C_IN = 32
C_OUT = 32
P_DIM = 64
Q = C_IN * C_OUT             # 1024
B, N = 32, 4096
N_CORES = 8
B_SH = B // N_CORES          # 4 batches per core
NPOS = B_SH * N              # 16384 positions per core
T = 512                      # positions per tile (matmul N)
TILES = NPOS // T            # 32
SG_TILES = 4                 # tiles per supergroup (col-tiled out^T group)
N_SG = TILES // SG_TILES     # 8
T_SG = T * SG_TILES          # 2048 positions per supergroup
PAIRS = 4                    # chunk pairs per tile (8 q'-chunks of 128)
# per chunk-pair-unit m-production path, indexed by unit_idx % 32:
#   D = ACT relu -> DVE tensor_tensor mult (2x bf16)
#   S = fused DVE scalar_tensor_tensor (relu+mult) straight from PSUM
UNIT_PATHS = os.environ.get(
    "K_PATHS", "SDDSDDDSDDSDDDSDDDSDDSDDDSDDSDDD"
)

F32 = mybir.dt.float32
BF16 = mybir.dt.bfloat16

_BUILD_CACHE = {}
LAST_RESULTS = None  # BassKernelResults of the most recent run (for profiling)


def _build_nc():
    nc = bacc.Bacc(
        "TRN2", target_bir_lowering=False, debug=False, num_devices=N_CORES
    )
    XR_d = nc.declare_dram_parameter("XR", [N_SG * 128, T_SG], BF16, isOutput=False)
    P2_d = nc.declare_dram_parameter("P2", [N_SG * 128, T_SG], BF16, isOutput=False)
    WK_d = nc.declare_dram_parameter("WK", [128, PAIRS * 128], BF16, isOutput=False)
    S_d = nc.declare_dram_parameter("S", [128, 8 * C_OUT], BF16, isOutput=False)
    out_d = nc.declare_dram_parameter("out", [N_SG * 128, T], BF16, isOutput=True)

    relu = mybir.ActivationFunctionType.Relu
    copyf = mybir.ActivationFunctionType.Copy
    mult = mybir.AluOpType.mult
    amax = mybir.AluOpType.max

    with ExitStack() as ctx:
        tc = ctx.enter_context(tile.TileContext(nc))
        wkp = ctx.enter_context(tc.tile_pool(name="wk", bufs=1))
        ssp = ctx.enter_context(tc.tile_pool(name="sel", bufs=1))
        xrp = ctx.enter_context(tc.tile_pool(name="xr", bufs=2))
        p2p = ctx.enter_context(tc.tile_pool(name="p2", bufs=2))
        apool = ctx.enter_context(tc.tile_pool(name="apsum", bufs=3, space="PSUM"))
        wpool = ctx.enter_context(tc.tile_pool(name="w", bufs=4))
        mpool = ctx.enter_context(tc.tile_pool(name="m", bufs=10))
        opool = ctx.enter_context(tc.tile_pool(name="opsum", bufs=2, space="PSUM"))
        obp = ctx.enter_context(tc.tile_pool(name="osb", bufs=2))

        wk_t = wkp.tile([128, PAIRS, 128], BF16)
        nc.sync.dma_start(
            out=wk_t[:], in_=WK_d[:].rearrange("p (a b) -> p a b", a=PAIRS)
        )
        s_t = ssp.tile([128, 8, C_OUT], BF16)
        nc.sync.dma_start(
            out=s_t[:], in_=S_d[:].rearrange("p (a b) -> p a b", a=8)
        )

        self_cnt = [0]
        for sg in range(N_SG):
            xr = xrp.tile([128, T_SG], BF16)
            p2 = p2p.tile([128, T_SG], BF16)
            # split loads along the free dim so tile j only waits on its
            # quarter; split p2 also by partition half (row-tiling operands)
            for fq in range(SG_TILES):
                fs = bass.ts(fq, T)
                r0 = sg * 128
                nc.sync.dma_start(out=xr[:, fs], in_=XR_d[r0:r0 + 128, fs])
                nc.sync.dma_start(
                    out=p2[0:64, fs], in_=P2_d[r0:r0 + 64, fs]
                )
                nc.sync.dma_start(
                    out=p2[64:128, fs], in_=P2_d[r0 + 64:r0 + 128, fs]
                )

            ot = opool.tile([128, T], F32)
            # SG-wide chunk-pair phases: all 4 tiles' units per pair, then
            # an 8-matmul reduce burst whose adjacent col-tiled matmuls
            # (4 col groups) run concurrently on the PE
            for p in range(PAIRS):
                units = []
                for j in range(SG_TILES):
                    js = bass.ts(j, T)
                    psA = apool.tile([128, 2, T], F32)
                    nc.tensor.matmul(
                        psA[:, 0, :], lhsT=wk_t[0:64, p, :],
                        rhs=p2[0:64, js], start=True, stop=True,
                    )
                    nc.tensor.matmul(
                        psA[:, 1, :], lhsT=wk_t[64:128, p, :],
                        rhs=p2[64:128, js], start=True, stop=True,
                    )
                    m = mpool.tile([128, 2, T], BF16)
                    x_in = xr[:, js].unsqueeze(1).broadcast_to(
                        [128, 2, T]
                    )
                    u = self_cnt[0] % len(UNIT_PATHS)
                    self_cnt[0] += 1
                    if UNIT_PATHS[u] == "S":
                        nc.vector.scalar_tensor_tensor(
                            out=m[:], in0=psA[:], scalar=0.0,
                            in1=x_in, op0=amax, op1=mult,
                        )
                    else:
                        w = wpool.tile([128, 2, T], BF16)
                        nc.scalar.activation(w[:], psA[:], relu)
                        nc.vector.tensor_tensor(
                            out=m[:], in0=w[:], in1=x_in, op=mult
                        )
                    units.append((j, m))
                for c2 in range(2):
                    chunk = 2 * p + c2
                    for (j, m) in units:
                        nc.tensor.matmul(
                            ot[32 * j:32 * (j + 1), :],
                            lhsT=s_t[:, chunk, :], rhs=m[:, c2, :],
                            start=(chunk == 0), stop=(chunk == 7),
                            tile_position=(0, 32 * j),
                        )

            osb = obp.tile([128, T], BF16)
            nc.scalar.activation(osb[:], ot[:], copyf)
            nc.sync.dma_start(
                out=out_d[sg * 128:(sg + 1) * 128, :], in_=osb[:]
            )

    nc.finalize()
    return nc


def _get_nc():
    key = "v6-" + UNIT_PATHS
    if key not in _BUILD_CACHE:
        _BUILD_CACHE[key] = _build_nc()
    return _BUILD_CACHE[key]


def _host_prep(X, P, Wk):
    """Build per-core input arrays (host-side prep is free)."""
    bf16 = ml_dtypes.bfloat16
    # Wk' with q' = o*32 + i
    WkP = np.ascontiguousarray(
        Wk.reshape(P_DIM, C_IN, C_OUT).transpose(0, 2, 1).reshape(P_DIM, Q)
    )
    # packed row-tiled pairs: [128, PAIRS, 128] -> [128, PAIRS*128]
    wk2 = np.zeros((128, PAIRS, 128), dtype=np.float32)
    for p in range(PAIRS):
        wk2[0:64, p, :] = WkP[:, 256 * p:256 * p + 128]
        wk2[64:128, p, :] = WkP[:, 256 * p + 128:256 * p + 256]
    WK_h = np.ascontiguousarray(wk2.reshape(128, PAIRS * 128)).astype(bf16)

    # S selection: S[pr, c, o] = 1 if o == 4c + pr//32
    pr = np.arange(128)[:, None, None]
    cc = np.arange(8)[None, :, None]
    oo = np.arange(C_OUT)[None, None, :]
    S = (oo == 4 * cc + pr // 32).astype(np.float32)
    S_h = np.ascontiguousarray(S.reshape(128, 8 * C_OUT)).astype(bf16)

    in_maps = []
    for c in range(N_CORES):
        Xc = np.ascontiguousarray(
            X[c * B_SH:(c + 1) * B_SH].reshape(NPOS, C_IN)
        )
        Pc = np.ascontiguousarray(
            P[c * B_SH:(c + 1) * B_SH].reshape(NPOS, P_DIM)
        )
        # X_rep [128, NPOS]: row pr = X[:, pr % 32]; then [sg] blocks
        XRc = np.tile(Xc.T, (4, 1))                    # [128, NPOS]
        XR_h = np.ascontiguousarray(
            XRc.reshape(128, N_SG, T_SG).transpose(1, 0, 2).reshape(
                N_SG * 128, T_SG
            )
        ).astype(bf16)
        # P^T duplicated x2 on partitions
        P2c = np.tile(Pc.T, (2, 1))                    # [128, NPOS]
        P2_h = np.ascontiguousarray(
            P2c.reshape(128, N_SG, T_SG).transpose(1, 0, 2).reshape(
                N_SG * 128, T_SG
            )
        ).astype(bf16)
        in_maps.append({"XR": XR_h, "P2": P2_h, "WK": WK_h, "S": S_h})
    return in_maps


def kernel(X, P, Wk):
    global LAST_RESULTS
    X = np.asarray(X, dtype=np.float32)
    P = np.asarray(P, dtype=np.float32)
    Wk = np.asarray(Wk, dtype=np.float32)

    in_maps = _host_prep(X, P, Wk)

    nc = _get_nc()
    trace = os.environ.get("BASS_PROFILE", "0") == "1"
    kw = {}
    if os.environ.get("BASS_TMPDIR"):
        kw["tmpdir"] = os.environ["BASS_TMPDIR"]
    res = run_bass_kernel_spmd(
        nc, in_maps, list(range(N_CORES)), trace=trace, **kw
    )
    LAST_RESULTS = res

    out = np.empty((B, N, C_OUT), dtype=np.float32)
    for c in range(N_CORES):
        # packed out^T: [sg*128 + 32j + o, t] -> pos = sg*T_SG + j*T + t
        o_c = (
            np.asarray(res.results[c]["out"])
            .astype(np.float32)
            .reshape(N_SG, SG_TILES, C_OUT, T)
            .transpose(0, 1, 3, 2)
            .reshape(B_SH, N, C_OUT)
        )
        out[c * B_SH:(c + 1) * B_SH] = o_c
    return out


# revision 16
# speedup vs baseline: 1.0631x; 1.0094x over previous
"""Trainium2 Bass kernel for nn_ConditionedDense (hypernetwork-conditioned dense).

Reference computation:
    A = einsum('bnp,pq->bnq', P, Wk)         # hypernetwork: per-position weights
    W = relu(A).reshape(B, N, c_in, c_out)
    out = einsum('bni,bnio->bno', X, W)

Strategy (v4): pure data parallel over 8 NeuronCores (shard batch dim),
A^T-oriented dataflow so both einsums run on the PE with static weights:

  - A^T layout: [q' partitions, pos free] with q' = o*32 + i.  PE computes
    A^T chunks (128 q' x T pos) with lhsT = Wk' chunk (static), rhs = P^T.
    K=64 -> two chunks run concurrently via row tiling (rows 0-63 / 64-127),
    with P^T duplicated on partitions 64-127.
  - m = relu(A) * X, per-tile path choice to balance ACT and DVE:
      D tile: 4x scalar.activation(Relu) PSUM->SBUF bf16 into one w8 tile,
              then ONE DVE tensor_tensor mult (2x bf16, FD=4096) by X
              replicated 4x on partitions (X_rep[p,t] = X[t, p%32]).
      S tile: 4x fused DVE scalar_tensor_tensor (max 0, mult) from PSUM.
  - reduce over i on the PE: 8 accumulating matmuls per tile with static
    0/1 selection weights S_c[p, o] = (o == 4c + p//32), output col-tiled
    into out^T PSUM [32j:32j+32, :].  Tiles are processed in PAIRS with
    their reduce matmuls emitted adjacently -> different col groups run
    concurrently on the PE array.
  - out stays in packed transposed layout; host unpacks (free).

Host side (free): P^T duplicated x2, X^T replicated x4, Wk column-permuted
to q' = o*32+i and packed into row-tiled pairs, S selection matrices, all
cast to bf16.
"""

import os
from contextlib import ExitStack

import numpy as np
import ml_dtypes

import concourse.bass as bass
import concourse.tile as tile
from concourse import bacc, mybir
from concourse.bass_utils import run_bass_kernel_spmd

C_IN = 32
C_OUT = 32
P_DIM = 64
Q = C_IN * C_OUT             # 1024
B, N = 32, 4096
N_CORES = 8
B_SH = B // N_CORES          # 4 batches per core
NPOS = B_SH * N              # 16384 positions per core
T = 512                      # positions per tile (matmul N)
TILES = NPOS // T            # 32
SG_TILES = 4                 # tiles per supergroup (col-tiled out^T group)
N_SG = TILES // SG_TILES     # 8
T_SG = T * SG_TILES          # 2048 positions per supergroup
PAIRS = 4                    # chunk pairs per tile (8 q'-chunks of 128)
# per chunk-pair-unit m-production path, indexed by unit_idx % 32:
#   D = ACT relu -> DVE tensor_tensor mult (2x bf16)
#   S = fused DVE scalar_tensor_tensor (relu+mult) straight from PSUM
UNIT_PATHS = os.environ.get(
    "K_PATHS", "SDDSDDDSDDSDDDSDDDSDDSDDDSDDSDDD"
)

F32 = mybir.dt.float32
BF16 = mybir.dt.bfloat16

_BUILD_CACHE = {}
LAST_RESULTS = None  # BassKernelResults of the most recent run (for profiling)


def _build_nc():
    nc = bacc.Bacc(
        "TRN2", target_bir_lowering=False, debug=False, num_devices=N_CORES
    )
    XR_d = nc.declare_dram_parameter("XR", [N_SG * 128, T_SG], BF16, isOutput=False)
    P2_d = nc.declare_dram_parameter("P2", [N_SG * 128, T_SG], BF16, isOutput=False)
    WK_d = nc.declare_dram_parameter("WK", [128, PAIRS * 128], BF16, isOutput=False)
    S_d = nc.declare_dram_parameter("S", [128, 8 * C_OUT], BF16, isOutput=False)
    out_d = nc.declare_dram_parameter("out", [N_SG * 128, T], BF16, isOutput=True)

    relu = mybir.ActivationFunctionType.Relu
    copyf = mybir.ActivationFunctionType.Copy
    mult = mybir.AluOpType.mult
    amax = mybir.AluOpType.max

    with ExitStack() as ctx:
        tc = ctx.enter_context(tile.TileContext(nc))
        wkp = ctx.enter_context(tc.tile_pool(name="wk", bufs=1))
        ssp = ctx.enter_context(tc.tile_pool(name="sel", bufs=1))
        xrp = ctx.enter_context(tc.tile_pool(name="xr", bufs=3))
        p2p = ctx.enter_context(tc.tile_pool(name="p2", bufs=3))
        apool = ctx.enter_context(tc.tile_pool(name="apsum", bufs=3, space="PSUM"))
        wpool = ctx.enter_context(tc.tile_pool(name="w", bufs=4))
        mpool = ctx.enter_context(tc.tile_pool(name="m", bufs=10))
        opool = ctx.enter_context(tc.tile_pool(name="opsum", bufs=2, space="PSUM"))
        obp = ctx.enter_context(tc.tile_pool(name="osb", bufs=2))

        wk_t = wkp.tile([128, PAIRS, 128], BF16)
        nc.sync.dma_start(
            out=wk_t[:], in_=WK_d[:].rearrange("p (a b) -> p a b", a=PAIRS)
        )
        s_t = ssp.tile([128, 8, C_OUT], BF16)
        nc.sync.dma_start(
            out=s_t[:], in_=S_d[:].rearrange("p (a b) -> p a b", a=8)
        )

        self_cnt = [0]
        pending_store = []
        for sg in range(N_SG):
            xr = xrp.tile([128, T_SG], BF16)
            p2 = p2p.tile([128, T_SG], BF16)
            # split loads along the free dim so tile j only waits on its
            # quarter; split p2 also by partition half (row-tiling operands)
            for fq in range(SG_TILES):
                fs = bass.ts(fq, T)
                r0 = sg * 128
                nc.sync.dma_start(out=xr[:, fs], in_=XR_d[r0:r0 + 128, fs])
                nc.sync.dma_start(
                    out=p2[0:64, fs], in_=P2_d[r0:r0 + 64, fs]
                )
                nc.sync.dma_start(
                    out=p2[64:128, fs], in_=P2_d[r0 + 64:r0 + 128, fs]
                )

            ot = opool.tile([128, T], F32)
            # SG-wide chunk-pair phases: all 4 tiles' units per pair, then
            # an 8-matmul reduce burst whose adjacent col-tiled matmuls
            # (4 col groups) run concurrently on the PE
            for p in range(PAIRS):
                units = []
                for j in range(SG_TILES):
                    js = bass.ts(j, T)
                    psA = apool.tile([128, 2, T], F32)
                    nc.tensor.matmul(
                        psA[:, 0, :], lhsT=wk_t[0:64, p, :],
                        rhs=p2[0:64, js], start=True, stop=True,
                    )
                    nc.tensor.matmul(
                        psA[:, 1, :], lhsT=wk_t[64:128, p, :],
                        rhs=p2[64:128, js], start=True, stop=True,
                    )
                    m = mpool.tile([128, 2, T], BF16)
                    x_in = xr[:, js].unsqueeze(1).broadcast_to(
                        [128, 2, T]
                    )
                    u = self_cnt[0] % len(UNIT_PATHS)
                    self_cnt[0] += 1
                    if UNIT_PATHS[u] == "S":
                        nc.vector.scalar_tensor_tensor(
                            out=m[:], in0=psA[:], scalar=0.0,
                            in1=x_in, op0=amax, op1=mult,
                        )
                    else:
                        w = wpool.tile([128, 2, T], BF16)
                        nc.scalar.activation(w[:], psA[:], relu)
                        nc.vector.tensor_tensor(
                            out=m[:], in0=w[:], in1=x_in, op=mult
                        )
                    units.append((j, m))
                for c2 in range(2):
                    chunk = 2 * p + c2
                    for (j, m) in units:
                        nc.tensor.matmul(
                            ot[32 * j:32 * (j + 1), :],
                            lhsT=s_t[:, chunk, :], rhs=m[:, c2, :],
                            start=(chunk == 0), stop=(chunk == 7),
                            tile_position=(0, 32 * j),
                        )

            pending_store.append((sg, ot))
            if len(pending_store) > 1:
                psg, pot = pending_store.pop(0)
                osb = obp.tile([128, T], BF16)
                nc.scalar.activation(osb[:], pot[:], copyf)
                nc.sync.dma_start(
                    out=out_d[psg * 128:(psg + 1) * 128, :], in_=osb[:]
                )
        for psg, pot in pending_store:
            osb = obp.tile([128, T], BF16)
            nc.scalar.activation(osb[:], pot[:], copyf)
            nc.sync.dma_start(
                out=out_d[psg * 128:(psg + 1) * 128, :], in_=osb[:]
            )

    nc.finalize()
    return nc


def _get_nc():
    key = "v7-" + UNIT_PATHS
    if key not in _BUILD_CACHE:
        _BUILD_CACHE[key] = _build_nc()
    return _BUILD_CACHE[key]


def _host_prep(X, P, Wk):
    """Build per-core input arrays (host-side prep is free)."""
    bf16 = ml_dtypes.bfloat16
    # Wk' with q' = o*32 + i
    WkP = np.ascontiguousarray(
        Wk.reshape(P_DIM, C_IN, C_OUT).transpose(0, 2, 1).reshape(P_DIM, Q)
    )
    # packed row-tiled pairs: [128, PAIRS, 128] -> [128, PAIRS*128]
    wk2 = np.zeros((128, PAIRS, 128), dtype=np.float32)
    for p in range(PAIRS):
        wk2[0:64, p, :] = WkP[:, 256 * p:256 * p + 128]
        wk2[64:128, p, :] = WkP[:, 256 * p + 128:256 * p + 256]
    WK_h = np.ascontiguousarray(wk2.reshape(128, PAIRS * 128)).astype(bf16)

    # S selection: S[pr, c, o] = 1 if o == 4c + pr//32
    pr = np.arange(128)[:, None, None]
    cc = np.arange(8)[None, :, None]
    oo = np.arange(C_OUT)[None, None, :]
    S = (oo == 4 * cc + pr // 32).astype(np.float32)
    S_h = np.ascontiguousarray(S.reshape(128, 8 * C_OUT)).astype(bf16)

    in_maps = []
    for c in range(N_CORES):
        Xc = np.ascontiguousarray(
            X[c * B_SH:(c + 1) * B_SH].reshape(NPOS, C_IN)
        )
        Pc = np.ascontiguousarray(
            P[c * B_SH:(c + 1) * B_SH].reshape(NPOS, P_DIM)
        )
        # X_rep [128, NPOS]: row pr = X[:, pr % 32]; then [sg] blocks
        XRc = np.tile(Xc.T, (4, 1))                    # [128, NPOS]
        XR_h = np.ascontiguousarray(
            XRc.reshape(128, N_SG, T_SG).transpose(1, 0, 2).reshape(
                N_SG * 128, T_SG
            )
        ).astype(bf16)
        # P^T duplicated x2 on partitions
        P2c = np.tile(Pc.T, (2, 1))                    # [128, NPOS]
        P2_h = np.ascontiguousarray(
            P2c.reshape(128, N_SG, T_SG).transpose(1, 0, 2).reshape(
                N_SG * 128, T_SG
            )
        ).astype(bf16)
        in_maps.append({"XR": XR_h, "P2": P2_h, "WK": WK_h, "S": S_h})
    return in_maps


def kernel(X, P, Wk):
    global LAST_RESULTS
    X = np.asarray(X, dtype=np.float32)
    P = np.asarray(P, dtype=np.float32)
    Wk = np.asarray(Wk, dtype=np.float32)

    in_maps = _host_prep(X, P, Wk)

    nc = _get_nc()
    trace = os.environ.get("BASS_PROFILE", "0") == "1"
    kw = {}
    if os.environ.get("BASS_TMPDIR"):
        kw["tmpdir"] = os.environ["BASS_TMPDIR"]
    res = run_bass_kernel_spmd(
        nc, in_maps, list(range(N_CORES)), trace=trace, **kw
    )
    LAST_RESULTS = res

    out = np.empty((B, N, C_OUT), dtype=np.float32)
    for c in range(N_CORES):
        # packed out^T: [sg*128 + 32j + o, t] -> pos = sg*T_SG + j*T + t
        o_c = (
            np.asarray(res.results[c]["out"])
            .astype(np.float32)
            .reshape(N_SG, SG_TILES, C_OUT, T)
            .transpose(0, 1, 3, 2)
            .reshape(B_SH, N, C_OUT)
        )
        out[c * B_SH:(c + 1) * B_SH] = o_c
    return out
